# revision 13
# baseline (speedup 1.0000x reference)
"""Trainium2 Bass kernel for nn_KeyRecorder.

Math (reference):
  comp = LN(relu(obs @ W1 + b1)) * g1 + bl1          [B, T, R]
  past = max(comp[:, :-20:10, :], axis=time)          408 strided rows
  gmax = max(cummax(comp[:, -20:, :]), past)          [B, 20, R]
  out  = LN(relu(gmax @ W2 + b2)) * g2 + bl2          [B, 20, D]

Only 428 of the 4096 timesteps per batch element are ever consumed
(408 strided + last 20), so the host gathers exactly those rows,
transposes them to d-major layout, and ships ~1.75 MB/core instead of
16.8 MB/core.  Batch is sharded 2-per-core across 8 cores (pure data
parallel, no collectives).

LN1's affine (g1, bl1) is folded into W2/b2 on the host:
  max/cummax commute with x -> x*g1+bl1 elementwise when g1 >= 0
  (asserted), and (gmax*g1+bl1) @ W2 = gmax @ (g1[:,None]*W2) + bl1@W2.
"""

import os
import numpy as np

import concourse.bass as bass
import concourse.bacc as bacc
import concourse.mybir as mybir
import concourse.tile as tile
from concourse.bass_utils import run_bass_kernel_spmd

F32 = mybir.dt.float32
ALU = mybir.AluOpType
ACT = mybir.ActivationFunctionType
AX = mybir.AxisListType

B, T, D, R = 16, 4096, 512, 64
LOCAL, SR, EPS = 20, 10, 1e-5
N_CORES = 8
BPC = B // N_CORES            # batch elements per core
NSTR = (T - LOCAL + SR - 1) // SR   # 408 strided past rows
NSEL = NSTR + LOCAL           # 428 rows consumed per batch element
GRP = 448                     # per-batch group width in SBUF (428 padded)
NTOK = GRP * BPC              # 896 token columns per core
NTT = NTOK // 128             # 7 token tiles
DC = D // 128                 # 4 contraction chunks
NO = BPC * LOCAL              # 40 output rows per core

IDX = np.array(list(range(0, T - LOCAL, SR)) + list(range(T - LOCAL, T)))

_cache: dict = {}


def _build_program():
    """Build + compile the per-core Bass program once."""
    if "nc" in _cache:
        return _cache["nc"]

    nc = bacc.Bacc("TRN2", target_bir_lowering=False, debug=False,
                   enable_asserts=False)

    obsT_d = nc.dram_tensor("obsT", [DC, 128, NTOK], F32, kind="ExternalInput")
    w1_d = nc.dram_tensor("w1c", [DC, 128, R], F32, kind="ExternalInput")
    b1_d = nc.dram_tensor("b1row", [1, R], F32, kind="ExternalInput")
    w2_d = nc.dram_tensor("w2f", [R, D], F32, kind="ExternalInput")
    b2_d = nc.dram_tensor("b2row", [1, D], F32, kind="ExternalInput")
    g2_d = nc.dram_tensor("g2b", [NO, D], F32, kind="ExternalInput")
    bl2_d = nc.dram_tensor("bl2b", [NO, D], F32, kind="ExternalInput")
    id_d = nc.dram_tensor("ident", [128, 128], F32, kind="ExternalInput")
    out_d = nc.dram_tensor("out", [NO, D], F32, kind="ExternalOutput")

    inv_r = 1.0 / R
    inv_d = 1.0 / D

    with tile.TileContext(nc) as tc:
        with (
            tc.tile_pool(name="const", bufs=1) as cpool,
            tc.tile_pool(name="work", bufs=3) as wpool,
            tc.tile_pool(name="stats", bufs=3) as spool,
            tc.tile_pool(name="ps_mm", bufs=4, space=bass.MemorySpace.PSUM) as pmm,
            tc.tile_pool(name="ps_tr", bufs=3, space=bass.MemorySpace.PSUM) as ptr,
            tc.tile_pool(name="ps_o", bufs=1, space=bass.MemorySpace.PSUM) as pout,
        ):
            # ---- load constants first (first matmul needs them) ----
            w1 = cpool.tile([128, DC, R], F32)
            for c in range(DC):
                nc.sync.dma_start(w1[:, c, :], w1_d[c])
            b1r = cpool.tile([1, R], F32)
            nc.sync.dma_start(b1r[:], b1_d[:])
            ident = cpool.tile([128, 128], F32)
            nc.sync.dma_start(ident[:], id_d[:])
            ones1 = cpool.tile([1, 128], F32)
            nc.vector.memset(ones1[:], 1.0)
            w2 = cpool.tile([R, D], F32)
            nc.sync.dma_start(w2[:], w2_d[:])
            b2r = cpool.tile([1, D], F32)
            nc.sync.dma_start(b2r[:], b2_d[:])
            g2 = cpool.tile([NO, D], F32)
            nc.sync.dma_start(g2[:], g2_d[:])
            bl2 = cpool.tile([NO, D], F32)
            nc.sync.dma_start(bl2[:], bl2_d[:])

            # ---- input: one tile per (chunk, token-tile) so each
            # token-tile's matmuls start as soon as its 4 slices land ----
            obst = [[None] * NTT for _ in range(DC)]
            for tt in range(NTT):
                for c in range(DC):
                    ot = cpool.tile([128, 128], F32, name=f"obst_{c}_{tt}")
                    nc.sync.dma_start(ot[:], obsT_d[c][:, bass.ts(tt, 128)])
                    obst[c][tt] = ot

            compT = cpool.tile([R, NTOK], F32)   # LN'd comp, [r, t] layout

            # ---- stage 1: comp = LN(relu(obs @ W1 + b1)) per 128-token tile
            for tt in range(NTT):
                ps = pmm.tile([128, R], F32, tag="ps")
                for c in range(DC):
                    nc.tensor.matmul(ps[:], obst[c][tt][:],
                                     w1[:, c, :], start=(c == 0), stop=False)
                # bias via K=1 ones-matmul: adds b1 to every row
                nc.tensor.matmul(ps[:], ones1[:], b1r[:], start=False, stop=True)

                # relu + row-sum in one op
                xr = wpool.tile([128, R], F32, tag="xr")
                rsum = spool.tile([128, 1], F32, tag="rsum")
                nc.vector.tensor_scalar(xr[:], ps[:], 0.0, 0.0, ALU.max,
                                        ALU.add, accum_out=rsum[:])
                negmu = spool.tile([128, 1], F32, tag="negmu")
                nc.gpsimd.tensor_scalar_mul(negmu[:], rsum[:], -inv_r)
                xc = wpool.tile([128, R], F32, tag="xc")
                nc.vector.tensor_scalar_add(xc[:], xr[:], negmu[:])
                # squared sum: square on ACT with fused row-sum
                sq = wpool.tile([128, R], F32, tag="sq")
                ssq = spool.tile([128, 1], F32, tag="ssq")
                nc.scalar.activation(sq[:], xc[:], ACT.Square,
                                     accum_out=ssq[:])
                ssqe = spool.tile([128, 1], F32, tag="ssqe")
                nc.vector.tensor_scalar_add(ssqe[:], ssq[:], R * EPS)
                std = spool.tile([128, 1], F32, tag="std")
                nc.scalar.activation(std[:], ssqe[:], ACT.Sqrt,
                                     bias=0.0, scale=inv_r)
                rstd = spool.tile([128, 1], F32, tag="rstd")
                nc.vector.reciprocal(rstd[:], std[:])
                y = wpool.tile([128, R], F32, tag="y")
                nc.vector.tensor_scalar_mul(y[:], xc[:], rstd[:])

                # transpose to [r, t] for the time reductions
                pt = ptr.tile([R, 128], F32, tag="pt")
                nc.tensor.transpose(pt[:], y[:], ident[:])
                nc.vector.tensor_copy(compT[:, bass.ts(tt, 128)], pt[:])

            # ---- stage 2: strided max + seeded cummax (free-axis ops) ----
            past0 = spool.tile([R, 1], F32, tag="past0")
            nc.vector.reduce_max(past0[:], compT[:, 0:NSTR], axis=AX.X)
            past1 = spool.tile([R, 1], F32, tag="past1")
            nc.vector.reduce_max(past1[:], compT[:, GRP:GRP + NSTR], axis=AX.X)

            pa = cpool.tile([R, BPC, LOCAL], F32)
            pb = cpool.tile([R, BPC, LOCAL], F32)
            nc.vector.tensor_copy(pa[:, 0, :], compT[:, NSTR:NSEL])
            nc.vector.tensor_copy(pa[:, 1, :], compT[:, GRP + NSTR:GRP + NSEL])
            cur, nxt = pa, pb
            s = 1
            while s < LOCAL:
                nc.vector.tensor_tensor(nxt[:, :, s:], cur[:, :, s:],
                                        cur[:, :, :LOCAL - s], op=ALU.max)
                nc.vector.tensor_copy(nxt[:, :, 0:s], cur[:, :, 0:s])
                cur, nxt = nxt, cur
                s *= 2

            gmaxT = cpool.tile([R, NO], F32)
            nc.vector.tensor_scalar(gmaxT[:, 0:LOCAL], cur[:, 0, :],
                                    past0[:], None, ALU.max)
            nc.vector.tensor_scalar(gmaxT[:, LOCAL:NO], cur[:, 1, :],
                                    past1[:], None, ALU.max)

            # ---- stage 3: out = LN(relu(gmax @ W2' + b2')) * g2 + bl2 ----
            ps2 = pout.tile([NO, D], F32)
            nc.tensor.matmul(ps2[:], gmaxT[:], w2[:], start=True, stop=False)
            nc.tensor.matmul(ps2[:], ones1[:, 0:NO], b2r[:],
                             start=False, stop=True)

            xr2 = cpool.tile([NO, D], F32)
            rsum2 = spool.tile([NO, 1], F32, tag="rsum2")
            nc.vector.tensor_scalar(xr2[:], ps2[:], 0.0, 0.0, ALU.max,
                                    ALU.add, accum_out=rsum2[:])
            negmu2 = spool.tile([NO, 1], F32, tag="negmu2")
            nc.gpsimd.tensor_scalar_mul(negmu2[:], rsum2[:], -inv_d)
            xc2 = cpool.tile([NO, D], F32)
            nc.vector.tensor_scalar_add(xc2[:], xr2[:], negmu2[:])
            sq2 = cpool.tile([NO, D], F32)
            ssq2 = spool.tile([NO, 1], F32, tag="ssq2")
            nc.scalar.activation(sq2[:], xc2[:], ACT.Square,
                                 accum_out=ssq2[:])
            ssqe2 = spool.tile([NO, 1], F32, tag="ssqe2")
            nc.vector.tensor_scalar_add(ssqe2[:], ssq2[:], D * EPS)
            std2 = spool.tile([NO, 1], F32, tag="std2")
            nc.scalar.activation(std2[:], ssqe2[:], ACT.Sqrt,
                                 bias=0.0, scale=inv_d)
            rstd2 = spool.tile([NO, 1], F32, tag="rstd2")
            nc.vector.reciprocal(rstd2[:], std2[:])
            yn = cpool.tile([NO, D], F32)
            nc.vector.tensor_scalar_mul(yn[:], xc2[:], rstd2[:])
            yg = cpool.tile([NO, D], F32)
            nc.vector.tensor_mul(yg[:], yn[:], g2[:])
            out_sb = cpool.tile([NO, D], F32)
            nc.vector.tensor_add(out_sb[:], yg[:], bl2[:])

            nc.sync.dma_start(out_d[:], out_sb[:])

    nc.compile()
    _cache["nc"] = nc
    return nc


def _host_inputs(obs, W1, b1, ln1_g, ln1_b, W2, b2, ln2_g, ln2_b):
    obs = np.ascontiguousarray(np.asarray(obs, dtype=np.float32))
    W1 = np.asarray(W1, np.float32)
    b1 = np.asarray(b1, np.float32)
    ln1_g = np.asarray(ln1_g, np.float32)
    ln1_b = np.asarray(ln1_b, np.float32)
    W2 = np.asarray(W2, np.float32)
    b2 = np.asarray(b2, np.float32)
    ln2_g = np.asarray(ln2_g, np.float32)
    ln2_b = np.asarray(ln2_b, np.float32)

    # folding LN1's affine past the max/cummax requires monotonicity
    assert np.all(ln1_g >= 0), "ln1_g must be >= 0 for the affine fold"

    w1c = np.ascontiguousarray(W1.reshape(DC, 128, R))
    b1r = b1.reshape(1, R)
    w2f = np.ascontiguousarray(ln1_g[:, None] * W2)
    b2f = (b2 + ln1_b @ W2).astype(np.float32).reshape(1, D)
    g2b = np.ascontiguousarray(np.broadcast_to(ln2_g, (NO, D)))
    bl2b = np.ascontiguousarray(np.broadcast_to(ln2_b, (NO, D)))
    ident = np.eye(128, dtype=np.float32)

    shared = {"w1c": w1c, "b1row": b1r, "w2f": w2f, "b2row": b2f,
              "g2b": g2b, "bl2b": bl2b, "ident": ident}
    in_maps = []
    for c in range(N_CORES):
        sel = obs[BPC * c:BPC * (c + 1)][:, IDX, :]        # [BPC, 428, 512]
        grp = np.zeros((BPC, GRP, D), np.float32)
        grp[:, :NSEL] = sel
        obsT = np.ascontiguousarray(grp.reshape(NTOK, D).T)  # [512, 896]
        in_maps.append({"obsT": obsT.reshape(DC, 128, NTOK), **shared})
    return in_maps


def _install_ntff_shim():
    """The agent image's antenv lacks axon_hooks; synthesize it so
    trace=True can reach the libaxon NTFF profiler (test-time only)."""
    import sys
    import types
    if "antenv.axon_hooks" in sys.modules:
        return True
    try:
        import antenv
        from trn_agent_boot.trn_boot import _ntff_profile_via_ctypes
    except ImportError:
        return False
    so_path = "/opt/axon/libaxon_pjrt.so"
    if not os.path.exists(so_path):
        return False
    hook = _ntff_profile_via_ctypes(so_path)
    mod = types.ModuleType("antenv.axon_hooks")
    mod._hook = hook
    mod.set_axon_ntff_profile_hook = lambda h: setattr(mod, "_hook", h)
    mod.get_axon_ntff_profile_hook = lambda: mod._hook
    sys.modules["antenv.axon_hooks"] = mod
    antenv.axon_hooks = mod
    return hook is not None


def kernel(obs_frames, W1, b1, ln1_g, ln1_b, W2, b2, ln2_g, ln2_b):
    nc = _build_program()
    in_maps = _host_inputs(obs_frames, W1, b1, ln1_g, ln1_b,
                           W2, b2, ln2_g, ln2_b)
    trace = bool(os.environ.get("BASS_TRACE"))
    if trace:
        trace = _install_ntff_shim()
        import concourse.bass_utils as _bu
        _bu.upload_artifacts = lambda tmpdir: f"local://{tmpdir}"
    res = run_bass_kernel_spmd(nc, in_maps, core_ids=list(range(N_CORES)),
                               trace=trace)
    _cache["last_result"] = res
    out = np.stack([res.results[c]["out"].reshape(BPC, LOCAL, D)
                    for c in range(N_CORES)])
    return out.reshape(B, LOCAL, D)


# revision 15
# speedup vs baseline: 1.0534x; 1.0534x over previous
"""Trainium2 Bass kernel for nn_KeyRecorder.

Math (reference):
  comp = LN(relu(obs @ W1 + b1)) * g1 + bl1          [B, T, R]
  past = max(comp[:, :-20:10, :], axis=time)          408 strided rows
  gmax = max(cummax(comp[:, -20:, :]), past)          [B, 20, R]
  out  = LN(relu(gmax @ W2 + b2)) * g2 + bl2          [B, 20, D]

Only 428 of the 4096 timesteps per batch element are ever consumed
(408 strided + last 20), so the host gathers exactly those rows,
transposes them to d-major layout, and ships ~1.75 MB/core instead of
16.8 MB/core.  Batch is sharded 2-per-core across 8 cores (pure data
parallel, no collectives).

LN1's affine (g1, bl1) is folded into W2/b2 on the host:
  max/cummax commute with x -> x*g1+bl1 elementwise when g1 >= 0
  (asserted), and (gmax*g1+bl1) @ W2 = gmax @ (g1[:,None]*W2) + bl1@W2.
"""

import os
import numpy as np

import concourse.bass as bass
import concourse.bacc as bacc
import concourse.mybir as mybir
import concourse.tile as tile
from concourse.bass_utils import run_bass_kernel_spmd

F32 = mybir.dt.float32
ALU = mybir.AluOpType
ACT = mybir.ActivationFunctionType
AX = mybir.AxisListType

B, T, D, R = 16, 4096, 512, 64
LOCAL, SR, EPS = 20, 10, 1e-5
N_CORES = 8
BPC = B // N_CORES            # batch elements per core
NSTR = (T - LOCAL + SR - 1) // SR   # 408 strided past rows
NSEL = NSTR + LOCAL           # 428 rows consumed per batch element
GRP = 448                     # per-batch group width in SBUF (428 padded)
NTOK = GRP * BPC              # 896 token columns per core
NTT = NTOK // 128             # 7 token tiles
DC = D // 128                 # 4 contraction chunks
NO = BPC * LOCAL              # 40 output rows per core

IDX = np.array(list(range(0, T - LOCAL, SR)) + list(range(T - LOCAL, T)))

_cache: dict = {}


def _build_program():
    """Build + compile the per-core Bass program once."""
    if "nc" in _cache:
        return _cache["nc"]

    nc = bacc.Bacc("TRN2", target_bir_lowering=False, debug=False,
                   enable_asserts=False)

    obsT_d = nc.dram_tensor("obsT", [DC, 128, NTOK], F32, kind="ExternalInput")
    w1_d = nc.dram_tensor("w1c", [DC, 128, R], F32, kind="ExternalInput")
    b1_d = nc.dram_tensor("b1row", [1, R], F32, kind="ExternalInput")
    w2_d = nc.dram_tensor("w2f", [R, D], F32, kind="ExternalInput")
    b2_d = nc.dram_tensor("b2row", [1, D], F32, kind="ExternalInput")
    g2_d = nc.dram_tensor("g2b", [NO, D], F32, kind="ExternalInput")
    bl2_d = nc.dram_tensor("bl2b", [NO, D], F32, kind="ExternalInput")
    id_d = nc.dram_tensor("ident", [128, 128], F32, kind="ExternalInput")
    out_d = nc.dram_tensor("out", [NO, D], F32, kind="ExternalOutput")

    inv_r = 1.0 / R
    inv_d = 1.0 / D

    with tile.TileContext(nc) as tc:
        with (
            tc.tile_pool(name="const", bufs=1) as cpool,
            tc.tile_pool(name="work", bufs=3) as wpool,
            tc.tile_pool(name="stats", bufs=3) as spool,
            tc.tile_pool(name="ps_mm", bufs=4, space=bass.MemorySpace.PSUM) as pmm,
            tc.tile_pool(name="ps_tr", bufs=3, space=bass.MemorySpace.PSUM) as ptr,
            tc.tile_pool(name="ps_o", bufs=1, space=bass.MemorySpace.PSUM) as pout,
        ):
            # ---- load constants first (first matmul needs them) ----
            w1 = cpool.tile([128, DC, R], F32)
            for c in range(DC):
                nc.sync.dma_start(w1[:, c, :], w1_d[c])
            b1r = cpool.tile([1, R], F32)
            nc.sync.dma_start(b1r[:], b1_d[:])
            ident = cpool.tile([128, 128], F32)
            nc.sync.dma_start(ident[:], id_d[:])
            ones1 = cpool.tile([1, 128], F32)
            nc.vector.memset(ones1[:], 1.0)
            w2 = cpool.tile([R, D], F32)
            nc.sync.dma_start(w2[:], w2_d[:])
            b2r = cpool.tile([1, D], F32)
            nc.sync.dma_start(b2r[:], b2_d[:])
            g2 = cpool.tile([NO, D], F32)
            nc.sync.dma_start(g2[:], g2_d[:])
            bl2 = cpool.tile([NO, D], F32)
            nc.sync.dma_start(bl2[:], bl2_d[:])

            # ---- input: two tiles per chunk (tiles 0-3 / 4-6) so the
            # first half's matmuls start after half the load ----
            obst = [[None, None] for _ in range(DC)]
            for h, (lo, w) in enumerate(((0, 512), (512, 384))):
                for c in range(DC):
                    ot = cpool.tile([128, w], F32, name=f"obst_{c}_{h}")
                    nc.sync.dma_start(ot[:], obsT_d[c][:, lo:lo + w])
                    obst[c][h] = ot

            compT = cpool.tile([R, NTOK], F32)   # LN'd comp, [r, t] layout

            # ---- stage 1: comp = LN(relu(obs @ W1 + b1)) per 128-token tile
            for tt in range(NTT):
                ps = pmm.tile([128, R], F32, tag="ps")
                h, off = (0, tt * 128) if tt < 4 else (1, (tt - 4) * 128)
                for c in range(DC):
                    nc.tensor.matmul(ps[:], obst[c][h][:, off:off + 128],
                                     w1[:, c, :], start=(c == 0), stop=False)
                # bias via K=1 ones-matmul: adds b1 to every row
                nc.tensor.matmul(ps[:], ones1[:], b1r[:], start=False, stop=True)

                # relu + row-sum in one op
                xr = wpool.tile([128, R], F32, tag="xr")
                rsum = spool.tile([128, 1], F32, tag="rsum")
                nc.vector.tensor_scalar(xr[:], ps[:], 0.0, 0.0, ALU.max,
                                        ALU.add, accum_out=rsum[:])
                negmu = spool.tile([128, 1], F32, tag="negmu")
                nc.gpsimd.tensor_scalar_mul(negmu[:], rsum[:], -inv_r)
                xc = wpool.tile([128, R], F32, tag="xc")
                nc.vector.tensor_scalar_add(xc[:], xr[:], negmu[:])
                # squared sum: square on ACT with fused row-sum
                sq = wpool.tile([128, R], F32, tag="sq")
                ssq = spool.tile([128, 1], F32, tag="ssq")
                nc.scalar.activation(sq[:], xc[:], ACT.Square,
                                     accum_out=ssq[:])
                ssqe = spool.tile([128, 1], F32, tag="ssqe")
                nc.vector.tensor_scalar_add(ssqe[:], ssq[:], R * EPS)
                std = spool.tile([128, 1], F32, tag="std")
                nc.scalar.activation(std[:], ssqe[:], ACT.Sqrt,
                                     bias=0.0, scale=inv_r)
                rstd = spool.tile([128, 1], F32, tag="rstd")
                nc.vector.reciprocal(rstd[:], std[:])
                y = wpool.tile([128, R], F32, tag="y")
                nc.vector.tensor_scalar_mul(y[:], xc[:], rstd[:])

                # transpose to [r, t] for the time reductions
                pt = ptr.tile([R, 128], F32, tag="pt")
                nc.tensor.transpose(pt[:], y[:], ident[:])
                nc.vector.tensor_copy(compT[:, bass.ts(tt, 128)], pt[:])

            # ---- stage 2: strided max + seeded cummax (free-axis ops) ----
            past0 = spool.tile([R, 1], F32, tag="past0")
            nc.vector.reduce_max(past0[:], compT[:, 0:NSTR], axis=AX.X)
            past1 = spool.tile([R, 1], F32, tag="past1")
            nc.vector.reduce_max(past1[:], compT[:, GRP:GRP + NSTR], axis=AX.X)

            pa = cpool.tile([R, BPC, LOCAL], F32)
            pb = cpool.tile([R, BPC, LOCAL], F32)
            nc.vector.tensor_copy(pa[:, 0, :], compT[:, NSTR:NSEL])
            nc.vector.tensor_copy(pa[:, 1, :], compT[:, GRP + NSTR:GRP + NSEL])
            cur, nxt = pa, pb
            s = 1
            while s < LOCAL:
                nc.vector.tensor_tensor(nxt[:, :, s:], cur[:, :, s:],
                                        cur[:, :, :LOCAL - s], op=ALU.max)
                nc.vector.tensor_copy(nxt[:, :, 0:s], cur[:, :, 0:s])
                cur, nxt = nxt, cur
                s *= 2

            gmaxT = cpool.tile([R, NO], F32)
            nc.vector.tensor_scalar(gmaxT[:, 0:LOCAL], cur[:, 0, :],
                                    past0[:], None, ALU.max)
            nc.vector.tensor_scalar(gmaxT[:, LOCAL:NO], cur[:, 1, :],
                                    past1[:], None, ALU.max)

            # ---- stage 3: out = LN(relu(gmax @ W2' + b2')) * g2 + bl2 ----
            ps2 = pout.tile([NO, D], F32)
            nc.tensor.matmul(ps2[:], gmaxT[:], w2[:], start=True, stop=False)
            nc.tensor.matmul(ps2[:], ones1[:, 0:NO], b2r[:],
                             start=False, stop=True)

            xr2 = cpool.tile([NO, D], F32)
            rsum2 = spool.tile([NO, 1], F32, tag="rsum2")
            nc.vector.tensor_scalar(xr2[:], ps2[:], 0.0, 0.0, ALU.max,
                                    ALU.add, accum_out=rsum2[:])
            negmu2 = spool.tile([NO, 1], F32, tag="negmu2")
            nc.gpsimd.tensor_scalar_mul(negmu2[:], rsum2[:], -inv_d)
            xc2 = cpool.tile([NO, D], F32)
            nc.vector.tensor_scalar_add(xc2[:], xr2[:], negmu2[:])
            sq2 = cpool.tile([NO, D], F32)
            ssq2 = spool.tile([NO, 1], F32, tag="ssq2")
            nc.scalar.activation(sq2[:], xc2[:], ACT.Square,
                                 accum_out=ssq2[:])
            ssqe2 = spool.tile([NO, 1], F32, tag="ssqe2")
            nc.vector.tensor_scalar_add(ssqe2[:], ssq2[:], D * EPS)
            std2 = spool.tile([NO, 1], F32, tag="std2")
            nc.scalar.activation(std2[:], ssqe2[:], ACT.Sqrt,
                                 bias=0.0, scale=inv_d)
            rstd2 = spool.tile([NO, 1], F32, tag="rstd2")
            nc.vector.reciprocal(rstd2[:], std2[:])
            yn = cpool.tile([NO, D], F32)
            nc.vector.tensor_scalar_mul(yn[:], xc2[:], rstd2[:])
            yg = cpool.tile([NO, D], F32)
            nc.vector.tensor_mul(yg[:], yn[:], g2[:])
            out_sb = cpool.tile([NO, D], F32)
            nc.vector.tensor_add(out_sb[:], yg[:], bl2[:])

            nc.sync.dma_start(out_d[:], out_sb[:])

    nc.compile()
    _cache["nc"] = nc
    return nc


def _host_inputs(obs, W1, b1, ln1_g, ln1_b, W2, b2, ln2_g, ln2_b):
    obs = np.ascontiguousarray(np.asarray(obs, dtype=np.float32))
    W1 = np.asarray(W1, np.float32)
    b1 = np.asarray(b1, np.float32)
    ln1_g = np.asarray(ln1_g, np.float32)
    ln1_b = np.asarray(ln1_b, np.float32)
    W2 = np.asarray(W2, np.float32)
    b2 = np.asarray(b2, np.float32)
    ln2_g = np.asarray(ln2_g, np.float32)
    ln2_b = np.asarray(ln2_b, np.float32)

    # folding LN1's affine past the max/cummax requires monotonicity
    assert np.all(ln1_g >= 0), "ln1_g must be >= 0 for the affine fold"

    w1c = np.ascontiguousarray(W1.reshape(DC, 128, R))
    b1r = b1.reshape(1, R)
    w2f = np.ascontiguousarray(ln1_g[:, None] * W2)
    b2f = (b2 + ln1_b @ W2).astype(np.float32).reshape(1, D)
    g2b = np.ascontiguousarray(np.broadcast_to(ln2_g, (NO, D)))
    bl2b = np.ascontiguousarray(np.broadcast_to(ln2_b, (NO, D)))
    ident = np.eye(128, dtype=np.float32)

    shared = {"w1c": w1c, "b1row": b1r, "w2f": w2f, "b2row": b2f,
              "g2b": g2b, "bl2b": bl2b, "ident": ident}
    in_maps = []
    for c in range(N_CORES):
        sel = obs[BPC * c:BPC * (c + 1)][:, IDX, :]        # [BPC, 428, 512]
        grp = np.zeros((BPC, GRP, D), np.float32)
        grp[:, :NSEL] = sel
        obsT = np.ascontiguousarray(grp.reshape(NTOK, D).T)  # [512, 896]
        in_maps.append({"obsT": obsT.reshape(DC, 128, NTOK), **shared})
    return in_maps


def _install_ntff_shim():
    """The agent image's antenv lacks axon_hooks; synthesize it so
    trace=True can reach the libaxon NTFF profiler (test-time only)."""
    import sys
    import types
    if "antenv.axon_hooks" in sys.modules:
        return True
    try:
        import antenv
        from trn_agent_boot.trn_boot import _ntff_profile_via_ctypes
    except ImportError:
        return False
    so_path = "/opt/axon/libaxon_pjrt.so"
    if not os.path.exists(so_path):
        return False
    hook = _ntff_profile_via_ctypes(so_path)
    mod = types.ModuleType("antenv.axon_hooks")
    mod._hook = hook
    mod.set_axon_ntff_profile_hook = lambda h: setattr(mod, "_hook", h)
    mod.get_axon_ntff_profile_hook = lambda: mod._hook
    sys.modules["antenv.axon_hooks"] = mod
    antenv.axon_hooks = mod
    return hook is not None


def kernel(obs_frames, W1, b1, ln1_g, ln1_b, W2, b2, ln2_g, ln2_b):
    nc = _build_program()
    in_maps = _host_inputs(obs_frames, W1, b1, ln1_g, ln1_b,
                           W2, b2, ln2_g, ln2_b)
    trace = bool(os.environ.get("BASS_TRACE"))
    if trace:
        trace = _install_ntff_shim()
        import concourse.bass_utils as _bu
        _bu.upload_artifacts = lambda tmpdir: f"local://{tmpdir}"
    res = run_bass_kernel_spmd(nc, in_maps, core_ids=list(range(N_CORES)),
                               trace=trace)
    _cache["last_result"] = res
    out = np.stack([res.results[c]["out"].reshape(BPC, LOCAL, D)
                    for c in range(N_CORES)])
    return out.reshape(B, LOCAL, D)
